# revision 45
# baseline (speedup 1.0000x reference)
"""CDVAE encoder GNN — Trainium2 Bass kernel (8-core data-parallel over graphs).

Key structure (validated against the reference in fp64 to ~3e-6):
 - The coordinate-update pathway is dead code for (mu, logvar): coords only
   feed future coord updates; dist/ea are computed once from the original
   coords.  So the per-edge coord MLP is skipped entirely.
 - Edges are the full i!=j set within each 32-atom graph -> everything is
   computed dense per graph (32x32 pairs) with the diagonal POISONED via an
   extra indicator row (h1_diag ~ -1e7 -> silu contribution == 0), which
   makes sum_j silu(h1[i,j]) directly equal the segment-sum over real edges.
 - Edge-MLP layer 1 decomposes: h1[i,j] = a[i] + b[j] + c[i,j] + b1 with
   a = node@Wa, b = node@Wb, c = ea@Wc.  a/b/poison enter the matmul
   through constant indicator matrices; b1 rides the silu bias operand.
 - segment-sum commutes with the linear edge_w2: nm = (sum_j s1) @ W2 + 31*b2.

Transfer plan (the dominant cost is host->device input upload):
 - All big weights/constants are packed host-side into ONE fp16 blob in
   SBUF-ready layout, sharded 1/8 per core, and AllGathered on device
   (small setup block + per-layer blocks so layer 0 arrives early).
   Small fp32 params ride a tiny fp32 blob.  Upload per core drops from
   ~39 MB to ~2.6 MB.
 - The whole layer pipeline runs on fp16 SBUF operands (PSUM stays fp32),
   so gathered weights feed matmuls directly with no cast pass.

Sharding: 16 graphs per core, no other inter-core communication.
"""

import math
import numpy as np

import concourse.bass as bass
import concourse.mybir as mybir
import concourse.tile as tile
from concourse import bacc
from concourse.bass import ds, ts
from concourse.masks import make_identity

F32 = mybir.dt.float32
F32R = mybir.dt.float32r
F16 = mybir.dt.float16
BF16 = mybir.dt.bfloat16
I32 = mybir.dt.int32
AF = mybir.ActivationFunctionType
ALU = mybir.AluOpType

G_TOT = 128      # graphs total
NA = 32          # atoms per graph
GPC = 16         # graphs per core
NPC = GPC * NA   # nodes per core (512)
H = 512
ED = 64
L = 6
LAT2 = 512       # 2*latent
NCORES = 8

TWO_PI = 2.0 * math.pi
CW_C1 = 6.28125                      # high bits of 2*pi (exact * small ints)
CW_C2 = float(np.float32(TWO_PI - CW_C1))
INV_2PI = 1.0 / TWO_PI
RNE_MAGIC = 1.5 * 2.0 ** 23          # fp32 round-to-nearest-int trick
POISON = -60000.0                    # fp16-representable; silu(-6e4) == 0

# Per-layer power-of-4 storage scales: activations grow ~10x per layer (to
# ~4e6 by L5) and would overflow fp16.  Stored tensors carry these exact
# exponent-shift scales; the inverse rides host-scaled weights / the silu
# input-scale operand, so no extra rounding error is introduced.
WSC = [1.0, 1.0, 1.0, 4.0 ** -1, 4.0 ** -2, 4.0 ** -3, 4.0 ** -4]  # node
YSC = [1.0, 1.0, 4.0 ** -1, 4.0 ** -2, 4.0 ** -3, 4.0 ** -5]       # nm
ZSC = [1.0, 1.0, 4.0 ** -1, 4.0 ** -2, 4.0 ** -3, 4.0 ** -5]       # ssub
VSC = [1.0, 1.0, 1.0, 4.0 ** -2, 4.0 ** -3, 4.0 ** -4]             # z1
QSC = 4.0 ** -4                                                     # q1

# ---- packed fp16 blob layout (element offsets) ----
HWE = 128 * 2048                 # one (512,512) matrix, SBUF-packed
# setup block: econst, elinw2 (block-diag doubled for the paired ea phase)
O_ECONST = 0
O_ELINW = O_ECONST + ED * NA * NA
S16S = O_ELINW + 128 * 128       # 81,920
# per-layer block: wa wb w2 wn1a wn1b wn2 wc
O_WA, O_WB, O_W2, O_WN1A, O_WN1B, O_WN2 = (i * HWE for i in range(6))
O_WC = 6 * HWE
LSTR16 = 6 * HWE + ED * H        # 1,605,632
GW16 = 2 * HWE                   # gw1 + gw2 block
SH16S = S16S // NCORES           # 15,104
SH16L = LSTR16 // NCORES         # 200,704
SH16G = GW16 // NCORES           # 65,536
SH16 = SH16S + L * SH16L + SH16G # 1,284,864 per-core fp16 elems

_o = 0
def _nxt(n):
    global _o
    r = _o
    _o += n
    return r

O32_AEMBED = _nxt(100 * H)       # first: indirect-DMA source needs offset 0
O32_GFPW = _nxt(2 * 128)         # doubled block layout for paired ea phase
O32_QSHIFT = _nxt(128)
O32_SINBIAS = _nxt(128)
O32_ELINB = _nxt(ED)
O32_B1S = _nxt(L * H)
O32_B2S = _nxt(L * H)
O32_NB1S = _nxt(L * H)
O32_NB2S = _nxt(L * H)
O32_GB1S = _nxt(H)
O32_GB2S = _nxt(H)
S32 = _o                         # 13,568
SH32 = S32 // NCORES             # 1,696
assert S32 % NCORES == 0 and S16S % NCORES == 0


def _r(ap):
    """bitcast an fp32 AP to float32r for full-rate matmul."""
    return ap.bitcast(F32R)


DEBUG_DUMPS = False


def build_module():
    """Build the per-core Bass module (same program on all 8 cores)."""
    nc = bacc.Bacc("TRN2", target_bir_lowering=False, debug=False,
                   num_devices=NCORES)

    # ---- DRAM tensors ----
    coords_t = nc.dram_tensor("coords_t", [3, NPC], F32, kind="ExternalInput").ap()
    atypes = nc.dram_tensor("atypes", [NPC], I32, kind="ExternalInput").ap()
    wsh16 = nc.dram_tensor("wsh16", [SH16], F16, kind="ExternalInput").ap()
    wsh32 = nc.dram_tensor("wsh32", [SH32], F32, kind="ExternalInput").ap()

    b16b = nc.dram_tensor("b16b", [SH16], F16)
    b32b = nc.dram_tensor("b32b", [SH32], F32)
    g16s = nc.dram_tensor("g16s", [NCORES, SH16S], F16, addr_space="Shared")
    g16l = [
        nc.dram_tensor(f"g16l{l}", [NCORES, SH16L], F16, addr_space="Shared")
        for l in range(L)
    ]
    g16g = nc.dram_tensor("g16g", [NCORES, SH16G], F16, addr_space="Shared")
    g32 = nc.dram_tensor("g32", [NCORES, SH32], F32, addr_space="Shared")

    lat_out = nc.dram_tensor("lat", [LAT2, GPC], F32, kind="ExternalOutput").ap()

    g16sf = g16s.ap().rearrange("a b -> (a b)")
    g16lf = [t.ap().rearrange("a b -> (a b)") for t in g16l]
    g16gf = g16g.ap().rearrange("a b -> (a b)")
    g32f = g32.ap().rearrange("a b -> (a b)")

    def v(flat, off, n, cols=None, rows=None):
        ap = flat[ds(off, n)]
        if cols is not None:
            return ap.rearrange("(p m) -> p m", m=cols)
        return ap.rearrange("(p m) -> p m", p=rows)

    econst = v(g16sf, O_ECONST, ED * NA * NA, cols=NA * NA)   # (64,1024) f16
    aembed = v(g32f, O32_AEMBED, 100 * H, cols=H)             # (100,512) f32
    elinw = v(g16sf, O_ELINW, 128 * 128, cols=128)            # (128,128) f16
    gfpw = v(g32f, O32_GFPW, 2 * 128, cols=128)               # (2,128)   f32
    qshift = v(g32f, O32_QSHIFT, 128, cols=1)                 # (128,1)
    sinbias = v(g32f, O32_SINBIAS, 128, cols=1)
    elinb = v(g32f, O32_ELINB, ED, cols=1)

    def bias_v(off, l=None):
        o = off + (0 if l is None else l * H)
        return v(g32f, o, H, cols=4)                          # (128,4) f32

    def w16_v(l, toff, n=HWE, cols=2048):
        return v(g16lf[l], toff, n, cols=cols)

    with tile.TileContext(nc) as tc:
        # ---------- input shards -> bounce -> AllGather ----------
        rg = [list(range(NCORES))]
        nc.gpsimd.dma_start(out=b32b.ap(), in_=wsh32)
        nc.gpsimd.dma_start(out=b16b.ap(), in_=wsh16)
        nc.gpsimd.collective_compute(
            "AllGather", ALU.bypass, replica_groups=rg,
            ins=[b16b.ap()[ds(0, SH16S)].opt()], outs=[g16s.ap().opt()],
        )
        nc.gpsimd.collective_compute(
            "AllGather", ALU.bypass, replica_groups=rg,
            ins=[b32b.ap().opt()], outs=[g32.ap().opt()],
        )
        for l in range(L):
            nc.gpsimd.collective_compute(
                "AllGather", ALU.bypass, replica_groups=rg,
                ins=[b16b.ap()[ds(SH16S + l * SH16L, SH16L)].opt()],
                outs=[g16l[l].ap().opt()],
            )
        nc.gpsimd.collective_compute(
            "AllGather", ALU.bypass, replica_groups=rg,
            ins=[b16b.ap()[ds(SH16S + L * SH16L, SH16G)].opt()],
            outs=[g16g.ap().opt()],
        )

        with (
            tc.tile_pool(name="cpool", bufs=1) as cpool,
            tc.tile_pool(name="npool", bufs=2) as npool,
            tc.tile_pool(name="pa", bufs=2, space="PSUM") as pa,
            tc.tile_pool(name="pb", bufs=3, space="PSUM") as pb,
        ):
            # ---------- persistent constants ----------
            # eac: rows 0:64 = eaT (per graph), rows 64:96 = delta_i,
            # rows 96:128 = delta_j (replicated per graph) -> phase-B rhs is
            # ONE stacked K=128 fp16 operand, 1024 pair-columns per graph.
            eac_sb = cpool.tile([128, GPC * NA * NA], F16, tag="eac")    # 32 KB/p
            gfpw_sb = cpool.tile([2, 128], F32, tag="gfpw")
            qshift_sb = cpool.tile([128, 1], F32, tag="qshift")
            sinbias_sb = cpool.tile([128, 1], F32, tag="sinbias")
            elinw_sb = cpool.tile([128, 128], F16, tag="elinw")
            elinb_sb = cpool.tile([ED, 1], F32, tag="elinb")
            ct_sb = cpool.tile([3, NPC], F32, tag="ct")
            ones3 = cpool.tile([3, NA], F32, tag="ones3")
            ident = cpool.tile([128, 128], F32, tag="ident")

            nc.sync.dma_start(
                out=eac_sb[ED:128, :].rearrange("p (g e) -> p g e", e=NA * NA),
                in_=econst.unsqueeze(1).broadcast_to([ED, GPC, NA * NA]),
            )
            nc.sync.dma_start(out=_r(gfpw_sb[:]), in_=_r(gfpw))
            nc.sync.dma_start(out=qshift_sb[:], in_=qshift)
            nc.sync.dma_start(out=sinbias_sb[:], in_=sinbias)
            nc.sync.dma_start(out=elinw_sb[:], in_=elinw)
            nc.sync.dma_start(out=elinb_sb[:], in_=elinb)
            nc.sync.dma_start(out=ct_sb[:], in_=coords_t)
            nc.vector.memset(ones3[:], 1.0)
            make_identity(nc, ident[:])

            _mark(nc, "setup")
            # ---------- setup: node gather + transpose ----------
            node0 = npool.tile([128, 4, H], F16, tag="node")
            with tc.tile_pool(name="spool", bufs=1) as spool:
                nrows = spool.tile([128, 4 * H], F32, tag="nrows")
                at_sb = spool.tile([128, 4], I32, tag="at")
                for t in range(4):
                    nc.sync.dma_start(
                        out=at_sb[:, t : t + 1],
                        in_=atypes[ds(t * 128, 128)].rearrange("(p o) -> p o", o=1),
                    )
                for t in range(4):
                    nc.gpsimd.indirect_dma_start(
                        out=nrows[:, ds(t * H, H)],
                        out_offset=None,
                        in_=aembed,
                        in_offset=bass.IndirectOffsetOnAxis(
                            ap=at_sb[:, t : t + 1], axis=0
                        ),
                    )
                # transpose 16 128x128 blocks: node0[hc*128+h, t*128+n] = node[n, h]
                for t in range(4):
                    for kc in range(4):
                        pt = pa.tile([128, 512], F32, tag="pa")
                        nc.tensor.transpose(
                            out=pt[:, 0:128],
                            in_=nrows[:, ds(t * H + kc * 128, 128)],
                            identity=ident[:],
                        )
                        nc.vector.tensor_copy(
                            out=node0[:, kc, ds(t * 128, 128)], in_=pt[:, 0:128]
                        )

                # ---------- setup: distances ----------
                # Gram construction in pair-partition layout:
                # d2pp[gg*32+i, q, j] = |c_i - c_j|^2 of graph g = q*4+gg
                # (sq[i] + sq[j] - 2<ci,cj> via three accumulated matmuls)
                ctm2 = spool.tile([3, NPC], F32, tag="ctm2")
                ctsq = spool.tile([3, NPC], F32, tag="ctsq")
                nc.vector.tensor_scalar(
                    out=ctm2[:], in0=ct_sb[:], scalar1=-2.0, scalar2=None,
                    op0=ALU.mult,
                )
                nc.vector.tensor_tensor(
                    out=ctsq[:], in0=ct_sb[:], in1=ct_sb[:], op=ALU.mult
                )
                d2pp = spool.tile([128, 4 * NA], F32, tag="d2pp")
                for q in range(4):
                    pd = pa.tile([128, 512], F32, tag="pa")
                    for gg in range(4):
                        g = q * 4 + gg
                        csl = ds(g * NA, NA)
                        tp = (0, 32 * gg) if gg else None
                        osl = pd[ds(32 * gg, 32), 0:NA]
                        nc.tensor.matmul(
                            out=osl, lhsT=ct_sb[:, csl], rhs=ctm2[:, csl],
                            start=True, stop=False, tile_position=tp,
                        )
                        nc.tensor.matmul(
                            out=osl, lhsT=ctsq[:, csl], rhs=ones3[:],
                            start=False, stop=False, tile_position=tp,
                        )
                        nc.tensor.matmul(
                            out=osl, lhsT=ones3[:], rhs=ctsq[:, csl],
                            start=False, stop=True, tile_position=tp,
                        )
                    # clamp: diagonal d2 is ~0 (may round slightly negative)
                    nc.vector.tensor_scalar(
                        out=d2pp[:, ds(q * NA, NA)], in0=pd[:, 0:NA], scalar1=1e-12,
                        scalar2=None, op0=ALU.max,
                    )
                s0 = spool.tile([128, 4 * NA], F32, tag="s0")
                nc.scalar.activation(s0[:], d2pp[:], AF.Sqrt)
                rr = spool.tile([128, 4 * NA], F32, tag="rr")
                nc.vector.reciprocal(out=rr[:], in_=s0[:])
                t1 = spool.tile([128, 4 * NA], F32, tag="t1")
                nc.vector.tensor_tensor(
                    out=t1[:], in0=d2pp[:], in1=rr[:], op=ALU.mult
                )
                # dsum = s0 + d2/s0 = 2*dist (Newton); the x0.5 is folded in gfpw
                dspp = spool.tile([128, 4 * NA], F32, tag="dspp")
                nc.vector.tensor_tensor(
                    out=dspp[:], in0=s0[:], in1=t1[:], op=ALU.add
                )

                # ---------- setup: ea (sin/cos features + linear) ----------
                # paired: both 16-row halves of a graph ride one 128-partition
                # pass (rows 0:64 = half 0 features, 64:128 = half 1)
                for g in range(GPC):
                    q, gg = g // 4, g % 4
                    dsl = spool.tile([2, 512], F32, tag="dsl")
                    nc.sync.dma_start(
                        out=_r(dsl[:]),
                        in_=_r(dspp[ds(gg * 32, 32), ds(q * NA, NA)]),
                    )
                    pxp = pa.tile([128, 512], F32, tag="pa")
                    nc.tensor.matmul(
                        out=pxp[:], lhsT=_r(gfpw_sb[:]),
                        rhs=_r(dsl[:]), start=True, stop=True,
                    )
                    tq = spool.tile([128, 512], F32, tag="tq")
                    nc.vector.tensor_scalar(
                        out=tq[:], in0=pxp[:],
                        scalar1=INV_2PI, scalar2=qshift_sb[:],
                        op0=ALU.mult, op1=ALU.add,
                    )
                    kk = spool.tile([128, 512], F32, tag="kk")
                    nc.vector.tensor_scalar(
                        out=kk[:], in0=tq[:],
                        scalar1=RNE_MAGIC, scalar2=RNE_MAGIC,
                        op0=ALU.add, op1=ALU.subtract,
                    )
                    v1 = spool.tile([128, 512], F32, tag="v1")
                    nc.vector.scalar_tensor_tensor(
                        out=v1[:], in0=kk[:], scalar=-CW_C1,
                        in1=pxp[:], op0=ALU.mult, op1=ALU.add,
                    )
                    xr = spool.tile([128, 512], F32, tag="xr")
                    nc.vector.scalar_tensor_tensor(
                        out=xr[:], in0=kk[:], scalar=-CW_C2,
                        in1=v1[:], op0=ALU.mult, op1=ALU.add,
                    )
                    ea0 = spool.tile([128, 512], F16, tag="ea0")
                    nc.scalar.activation(
                        ea0[:], xr[:], AF.Sin, bias=sinbias_sb[:], scale=1.0
                    )
                    for h2 in range(2):
                        pel = pa.tile([128, 512], F32, tag="pa")
                        nc.tensor.matmul(
                            out=pel[0:ED, :],
                            lhsT=elinw_sb[:, ds(h2 * ED, ED)], rhs=ea0[:],
                            start=True, stop=True,
                        )
                        nc.scalar.activation(
                            eac_sb[0:ED, ds(g * 1024 + h2 * 512, 512)],
                            pel[0:ED, :], AF.Identity,
                            bias=elinb_sb[:], scale=1.0,
                        )

            # ---------- layer-phase pools (opened after setup scratch frees) ----------
            from contextlib import ExitStack as _ES

            lctx = _ES()
            wpool = lctx.enter_context(tc.tile_pool(name="wpool", bufs=3))
            wsm = lctx.enter_context(tc.tile_pool(name="wsm", bufs=2))
            work = lctx.enter_context(tc.tile_pool(name="work", bufs=2))

            def load16(src):
                """DMA one packed fp16 (512,512) matrix; view [128,4,512]."""
                t16 = wpool.tile([128, 2048], F16, tag="w16")
                nc.sync.dma_start(out=t16[:], in_=src)
                return t16[:].rearrange("p (c m) -> p c m", m=512)

            # ---------- persistent layer-loop tensors ----------
            # wcab: phase-B stationary [Wc(64); a_g(32); b_g(32)] per graph
            wcab = cpool.tile([128, GPC, H], F16, tag="wcab")
            ssub32 = cpool.tile([128, 4, H], F32, tag="ssub32")
            ssub = cpool.tile([128, 4, H], F16, tag="ssub")
            nm_sb = cpool.tile([128, 4, H], F16, tag="nm")
            z1s = cpool.tile([128, 4, H], F16, tag="z1s")

            if DEBUG_DUMPS:
                dbg_eac = nc.dram_tensor("dbg_eac", [128, GPC * NA * NA], F16,
                                         kind="ExternalOutput").ap()
                dbg_node0 = nc.dram_tensor("dbg_node0", [128, 4 * H], F16,
                                           kind="ExternalOutput").ap()
                dbg_wcab = nc.dram_tensor("dbg_wcab", [128, GPC * H], F16,
                                          kind="ExternalOutput").ap()
                dbg_ssub = nc.dram_tensor("dbg_ssub", [128, 4 * H], F16,
                                          kind="ExternalOutput").ap()
                dbg_node1 = nc.dram_tensor("dbg_node1", [128, 4 * H], F16,
                                           kind="ExternalOutput").ap()

            node = node0
            for l in range(L):
                _mark(nc, f"L{l}.A")
                # ---- phase A: a/b projections (row layout, per graph) ----
                wa_t = load16(w16_v(l, O_WA))
                wb_t = load16(w16_v(l, O_WB))

                # Wc -> wcab rows 0:64 (replicated per graph)
                nc.sync.dma_start(
                    out=wcab[0:ED, :, :],
                    in_=w16_v(l, O_WC, ED * H, cols=H)
                    .unsqueeze(1).broadcast_to([ED, GPC, H]),
                )
                for t in range(4):  # 4 graphs per batch (M=128)
                    pab = pa.tile([128, 512], F32, tag="pa")
                    for kc in range(4):
                        nc.tensor.matmul(
                            out=pab[:],
                            lhsT=node[:, kc, ds(t * 128, 128)],
                            rhs=wa_t[:, kc, :],
                            start=(kc == 0), stop=(kc == 3),
                        )
                    pbt = pa.tile([128, 512], F32, tag="pa")
                    for kc in range(4):
                        nc.tensor.matmul(
                            out=pbt[:],
                            lhsT=node[:, kc, ds(t * 128, 128)],
                            rhs=wb_t[:, kc, :],
                            start=(kc == 0), stop=(kc == 3),
                        )
                    sta = work.tile([128, 512], F16, tag="stg")
                    nc.vector.tensor_copy(out=sta[:], in_=pab[:])
                    stb = work.tile([128, 512], F16, tag="stg")
                    nc.vector.tensor_copy(out=stb[:], in_=pbt[:])
                    for gg in range(4):
                        g = t * 4 + gg
                        nc.sync.dma_start(
                            out=wcab[ED : ED + NA, g, :],
                            in_=sta[ds(gg * NA, NA), :],
                        )
                        nc.sync.dma_start(
                            out=wcab[ED + NA : 128, g, :],
                            in_=stb[ds(gg * NA, NA), :],
                        )

                if DEBUG_DUMPS and l == 0:
                    nc.sync.dma_start(out=dbg_eac, in_=eac_sb[:])
                    nc.sync.dma_start(
                        out=dbg_node0,
                        in_=node[:].rearrange("p a b -> p (a b)"),
                    )
                    nc.sync.dma_start(
                        out=dbg_wcab,
                        in_=wcab[:].rearrange("p a b -> p (a b)"),
                    )

                _mark(nc, f"L{l}.B")
                # ---- phase B: dense edge pass (one 1024-col matmul per hc) ----
                b1s_t = wsm.tile([128, 4], F32, tag="bias")
                nc.sync.dma_start(out=b1s_t[:], in_=bias_v(O32_B1S, l))
                for g in range(GPC):
                    for hc in range(4):
                        ph = pb.tile([128, 1024], F32, tag="pb")
                        for h in range(2):
                            nc.tensor.matmul(
                                out=ph[:, ds(h * 512, 512)],
                                lhsT=wcab[:, g, ds(hc * 128, 128)],
                                rhs=eac_sb[:, ds(g * 1024 + h * 512, 512)],
                                start=True, stop=True,
                            )
                        # poison the (i==j) diagonal: silu(-6e4) == 0
                        nc.vector.memset(ph[:, 0 : NA * NA : NA + 1], POISON)
                        sg = work.tile([128, 1024], BF16, tag="sg")
                        nc.scalar.activation(
                            sg[:], ph[:], AF.Silu,
                            bias=b1s_t[:, hc : hc + 1], scale=1.0 / WSC[l],
                        )
                        nc.vector.tensor_reduce(
                            out=ssub32[:, hc, ds(g * NA, NA)],
                            in_=sg[:].rearrange("p (i j) -> p i j", j=NA),
                            op=ALU.add, axis=mybir.AxisListType.X,
                        )

                if DEBUG_DUMPS and l == 0:
                    nc.sync.dma_start(
                        out=dbg_ssub,
                        in_=ssub[:].rearrange("p a b -> p (a b)"),
                    )

                _mark(nc, f"L{l}.C")
                # ---- phase C: nm + node MLP ----
                for hc in range(4):
                    with nc.allow_low_precision(reason="fp16 pipeline"):
                        nc.vector.tensor_scalar(
                            out=ssub[:, hc, :], in0=ssub32[:, hc, :],
                            scalar1=ZSC[l], scalar2=None, op0=ALU.mult,
                        )
                w2_t = load16(w16_v(l, O_W2))
                b2s_t = wsm.tile([128, 4], F32, tag="bias")
                nc.sync.dma_start(out=b2s_t[:], in_=bias_v(O32_B2S, l))
                for hc in range(4):
                    pn = pa.tile([128, 512], F32, tag="pa")
                    for kc in range(4):
                        nc.tensor.matmul(
                            out=pn[:],
                            lhsT=w2_t[:, kc, ds(hc * 128, 128)],
                            rhs=ssub[:, kc, :],
                            start=(kc == 0), stop=(kc == 3),
                        )
                    nc.scalar.activation(
                        nm_sb[:, hc, :], pn[:], AF.Identity,
                        bias=b2s_t[:, hc : hc + 1], scale=1.0,
                    )

                wn1a_t = load16(w16_v(l, O_WN1A))
                wn1b_t = load16(w16_v(l, O_WN1B))
                nb1s_t = wsm.tile([128, 4], F32, tag="bias")
                nc.sync.dma_start(out=nb1s_t[:], in_=bias_v(O32_NB1S, l))
                for hc in range(4):
                    pz = pa.tile([128, 512], F32, tag="pa")
                    for kc in range(4):
                        nc.tensor.matmul(
                            out=pz[:],
                            lhsT=wn1a_t[:, kc, ds(hc * 128, 128)],
                            rhs=node[:, kc, :],
                            start=(kc == 0), stop=False,
                        )
                    for kc in range(4):
                        nc.tensor.matmul(
                            out=pz[:],
                            lhsT=wn1b_t[:, kc, ds(hc * 128, 128)],
                            rhs=nm_sb[:, kc, :],
                            start=False, stop=(kc == 3),
                        )
                    z132 = work.tile([128, 512], F32, tag="z132")
                    nc.scalar.activation(
                        z132[:], pz[:], AF.Silu,
                        bias=nb1s_t[:, hc : hc + 1], scale=1.0,
                    )
                    with nc.allow_low_precision(reason="fp16 pipeline"):
                        nc.vector.tensor_scalar(
                            out=z1s[:, hc, :], in0=z132[:],
                            scalar1=VSC[l], scalar2=None, op0=ALU.mult,
                        )

                wn2_t = load16(w16_v(l, O_WN2))
                nb2s_t = wsm.tile([128, 4], F32, tag="bias")
                nc.sync.dma_start(out=nb2s_t[:], in_=bias_v(O32_NB2S, l))
                node_next = npool.tile([128, 4, H], F16, tag="node")
                for hc in range(4):
                    pz2 = pa.tile([128, 512], F32, tag="pa")
                    for kc in range(4):
                        nc.tensor.matmul(
                            out=pz2[:],
                            lhsT=wn2_t[:, kc, ds(hc * 128, 128)],
                            rhs=z1s[:, kc, :],
                            start=(kc == 0), stop=(kc == 3),
                        )
                    nc.scalar.activation(
                        node_next[:, hc, :], pz2[:], AF.Identity,
                        bias=nb2s_t[:, hc : hc + 1], scale=1.0,
                    )
                node = node_next
                if DEBUG_DUMPS and l == 0:
                    nc.sync.dma_start(
                        out=dbg_node1,
                        in_=node[:].rearrange("p a b -> p (a b)"),
                    )

            _mark(nc, "final")
            # ---------- final: graph pool + latent MLP ----------
            graph_t = cpool.tile([128, 4, GPC], F32, tag="graph")
            for hc in range(4):
                with nc.allow_low_precision(reason="f32r round on write"):
                    nc.vector.tensor_reduce(
                        out=_r(graph_t[:, hc, :]),
                        in_=node[:, hc, :].rearrange("p (g a) -> p g a", a=NA),
                        op=ALU.add, axis=mybir.AxisListType.X,
                    )
            gw1_16 = load16(v(g16gf, 0, HWE, cols=2048))
            gw1_t = wpool.tile([128, 4, H], F32, tag="gw1f32")
            nc.vector.tensor_copy(out=_r(gw1_t[:]), in_=gw1_16)
            gb1s_t = wsm.tile([128, 4], F32, tag="bias")
            nc.sync.dma_start(out=gb1s_t[:], in_=bias_v(O32_GB1S))
            q1s = cpool.tile([128, 4, GPC], F16, tag="q1s")
            q132 = cpool.tile([128, 4, GPC], F32, tag="q132")
            for hc in range(4):
                pq = pa.tile([128, 512], F32, tag="pa")
                for kc in range(4):
                    nc.tensor.matmul(
                        out=pq[:, 0:GPC],
                        lhsT=_r(gw1_t[:, kc, ds(hc * 128, 128)]),
                        rhs=_r(graph_t[:, kc, :]),
                        start=(kc == 0), stop=(kc == 3),
                    )
                nc.scalar.activation(
                    q132[:, hc, :], pq[:, 0:GPC], AF.Silu,
                    bias=gb1s_t[:, hc : hc + 1], scale=1.0,
                )
            with nc.allow_low_precision(reason="fp16 pipeline"):
                nc.vector.tensor_scalar(
                    out=q1s[:].rearrange("p a b -> p (a b)"),
                    in0=q132[:].rearrange("p a b -> p (a b)"),
                    scalar1=QSC, scalar2=None, op0=ALU.mult,
                )

            gw2_t = load16(v(g16gf, HWE, HWE, cols=2048))
            gb2s_t = wsm.tile([128, 4], F32, tag="bias")
            nc.sync.dma_start(out=gb2s_t[:], in_=bias_v(O32_GB2S))
            lat_sb = cpool.tile([128, 4, GPC], F32, tag="lat")
            for oc in range(4):
                pl = pa.tile([128, 512], F32, tag="pa")
                for kc in range(4):
                    nc.tensor.matmul(
                        out=pl[:, 0:GPC],
                        lhsT=gw2_t[:, kc, ds(oc * 128, 128)],
                        rhs=q1s[:, kc, :],
                        start=(kc == 0), stop=(kc == 3),
                    )
                nc.scalar.activation(
                    lat_sb[:, oc, :], pl[:, 0:GPC], AF.Identity,
                    bias=gb2s_t[:, oc : oc + 1], scale=1.0,
                )
            nc.sync.dma_start(
                out=lat_out.rearrange("(c p) g -> p c g", p=128), in_=lat_sb[:]
            )
            lctx.close()

    nc.compile()
    return nc


def _pack6(m):
    """(512,512) fp32 -> (128, 2048) fp16 in SBUF partition-major layout."""
    return np.ascontiguousarray(
        m.reshape(4, 128, 512).transpose(1, 0, 2).reshape(128, 2048)
    ).astype(np.float16)


def prep_inputs(inputs):
    """Host-side packing: blobs + per-core shards."""
    f32 = np.float32
    f16 = np.float16
    coords = np.asarray(inputs["coords"], f32)
    atom_types = np.asarray(inputs["atom_types"], np.int32)
    ew1 = np.asarray(inputs["edge_w1"], f32)
    eb1 = np.asarray(inputs["edge_b1"], f32)

    def chunk_bias(b):  # (L?,512) -> (...,128,4) per-partition chunks
        b = np.asarray(b, f32)
        if b.ndim == 1:
            return np.ascontiguousarray(b.reshape(4, 128).T)
        return np.ascontiguousarray(b.reshape(-1, 4, 128).transpose(0, 2, 1))

    # ---- fp16 blob: setup block, per-layer blocks, gw block ----
    w2_ = np.asarray(inputs["edge_w2"], f32)
    wn1 = np.asarray(inputs["node_w1"], f32)
    wn2_ = np.asarray(inputs["node_w2"], f32)

    # indicator matrix: rows 0-31 delta(i), rows 32-63 delta(j)
    ec = np.zeros((ED, NA * NA), f16)
    ii, jj = np.meshgrid(np.arange(NA), np.arange(NA), indexing="ij")
    ii, jj = ii.ravel(), jj.ravel()
    ec[ii, np.arange(NA * NA)] = 1.0
    ec[32 + jj, np.arange(NA * NA)] = 1.0

    S16TOT = S16S + L * LSTR16 + GW16
    blob16 = np.empty(S16TOT, f16)
    blob16[O_ECONST : O_ECONST + ED * NA * NA] = ec.ravel()
    elinw2 = np.zeros((128, 128), f16)
    elw = np.asarray(inputs["edge_lin_w"], f16)
    elinw2[:ED, :ED] = elw
    elinw2[ED:, ED:] = elw
    blob16[O_ELINW : O_ELINW + 128 * 128] = elinw2.ravel()
    for l in range(L):
        base = S16S + l * LSTR16
        for off, m in (
            (O_WA, ew1[l, :H, :]),
            (O_WB, ew1[l, H : 2 * H, :]),
            (O_W2, w2_[l] * f32(YSC[l] / ZSC[l])),
            (O_WN1A, wn1[l, :H, :] * f32(1.0 / WSC[l])),
            (O_WN1B, wn1[l, H:, :] * f32(1.0 / YSC[l])),
            (O_WN2, wn2_[l] * f32(WSC[l + 1] / VSC[l])),
        ):
            blob16[base + off : base + off + HWE] = _pack6(m).ravel()
        blob16[base + O_WC : base + O_WC + ED * H] = (
            (ew1[l, 2 * H :, :] * f32(WSC[l])).astype(f16).ravel()
        )
    gbase = S16S + L * LSTR16
    blob16[gbase : gbase + HWE] = _pack6(
        np.asarray(inputs["graph_w1"], f32) / f32(NA * WSC[L])
    ).ravel()
    blob16[gbase + HWE : gbase + GW16] = _pack6(
        np.asarray(inputs["graph_w2"], f32) / f32(QSC)
    ).ravel()

    # ---- fp32 blob ----
    gfp = np.asarray(inputs["gfp_W"], f32)
    blob32 = np.empty(S32, f32)

    def put(off, arr):
        a = np.ascontiguousarray(np.asarray(arr, f32)).ravel()
        blob32[off : off + a.size] = a

    put(O32_AEMBED, inputs["atom_embed"])
    gfpw1 = np.concatenate([gfp, gfp]) * np.float32(math.pi)
    gfpw2 = np.zeros((2, 128), f32)
    gfpw2[0, :ED] = gfpw1
    gfpw2[1, ED:] = gfpw1
    put(O32_GFPW, gfpw2)
    qs1 = np.concatenate([np.zeros(32, f32), np.full(32, 0.25, f32)])
    put(O32_QSHIFT, np.concatenate([qs1, qs1]))
    sb1 = np.concatenate([np.zeros(32, f32), np.full(32, math.pi / 2, f32)])
    put(O32_SINBIAS, np.concatenate([sb1, sb1]))
    put(O32_ELINB, inputs["edge_lin_b"])
    put(O32_B1S, chunk_bias(eb1))
    put(O32_B2S, chunk_bias(
        np.asarray(inputs["edge_b2"], f32) * (NA - 1)
        * np.asarray(YSC, f32)[:, None]
    ))
    put(O32_NB1S, chunk_bias(inputs["node_b1"]))
    put(O32_NB2S, chunk_bias(
        np.asarray(inputs["node_b2"], f32)
        * np.asarray(WSC[1 : L + 1], f32)[:, None]
    ))
    put(O32_GB1S, chunk_bias(inputs["graph_b1"]))
    put(O32_GB2S, chunk_bias(inputs["graph_b2"]))

    # per-core fp16 shard: setup slice, per-layer slices, gw slice
    in_maps = []
    for c in range(NCORES):
        sl = slice(c * NPC, (c + 1) * NPC)
        sh16 = np.empty(SH16, f16)
        sh16[:SH16S] = blob16[c * SH16S : (c + 1) * SH16S]
        for l in range(L):
            b = S16S + l * LSTR16
            sh16[SH16S + l * SH16L : SH16S + (l + 1) * SH16L] = blob16[
                b + c * SH16L : b + (c + 1) * SH16L
            ]
        sh16[SH16S + L * SH16L :] = blob16[
            gbase + c * SH16G : gbase + (c + 1) * SH16G
        ]
        m = {
            "coords_t": np.ascontiguousarray(coords[sl].T),
            "atypes": np.ascontiguousarray(atom_types[sl]),
            "wsh16": sh16,
            "wsh32": np.ascontiguousarray(blob32[c * SH32 : (c + 1) * SH32]),
        }
        in_maps.append(m)
    return in_maps


_CACHE = {}
PHASE_MARKS = []


def _mark(nc, name):
    PHASE_MARKS.append((name, nc.next_id()))


def kernel(**inputs):
    from concourse import bass_utils

    if "nc" not in _CACHE:
        _CACHE["nc"] = build_module()
    nc = _CACHE["nc"]
    in_maps = prep_inputs(inputs)
    res = bass_utils.run_bass_kernel_spmd(
        nc, in_maps, core_ids=list(range(NCORES))
    )
    lat = np.concatenate(
        [res.results[c]["lat"].T for c in range(NCORES)], axis=0
    )  # (128, 512)
    mu, logvar = lat[:, : LAT2 // 2], lat[:, LAT2 // 2 :]
    return (mu, logvar)
